# revision 60
# baseline (speedup 1.0000x reference)
"""Trainium2 Bass kernel: multi-head attention (transposed-causal softmax).

Reference math (B=4, N=2048, D=1024, H=16, E=64):
    qkv = x @ W_qkv -> split (3, H, E)
    scores[i, j] = k_i . q_j / sqrt(E)          (i = key pos, j = query pos)
    mask: keep i <= j; softmax over j; out[i] = sum_j attn[i, j] v_j
    y = concat_heads(out) @ W_o
Sharding (8 cores): data-parallel over batch (4) x tensor-parallel over
head-groups (2 groups of 8 heads); the host sums the two partial
projections per batch.

Per-core dataflow (v7, CoreSim ~216us vs ~279us for the v2 baseline):
  - xt [D, N] supplied transposed so projections contract D on partitions.
  - Inputs stream in consumption order as ONE batched 3D-AP DMA per
    (tensor, priority group) -- the SP sequencer's ~565ns per-DMA issue
    cost made many small DMAs the startup bottleneck.
  - V is projected per head pair into vna [128, NT*520]: per j-tile,
    8 x [64 V cols | 1 ones col]; the ones column makes the AV matmul
    emit the softmax denominator Z alongside O.  Pair 0's V tiles are
    computed just-in-time; pairs 1-3's V are deferred PE filler.
  - i-chunks are 512 wide.  Scores land in [128, 2x512] PSUM (one BANK
    per head -- a matmul writing a non-bank-aligned PSUM offset with
    tile_position (64, 0) faults on real hardware), double-buffered;
    one exp per (chunk, j-tile) covers both heads.
  - AV runs in NATURAL layout: per 128-row i-tile, lhsT = exp-scores
    slice [j, i-tile], rhs = vna block [j, 65] -> po[i, 65] accumulated
    over j-tiles (65-col streams instead of w<=512 per head, ~2x less PE
    than the O^T form).  PSUM zero-region discipline: only the first
    matmul into a po bank uses start=True; a start=True per slice would
    re-mark earlier slices' bytes pending-zero and drop them.
  - Normalization per i-tile as soon as its diagonal j-tile stops:
    reciprocal_approx_fast on the Z column, per-partition
    tensor_scalar_mul -> o_nat [128, 128] bf16 (2 heads), then a DMA
    XBAR transpose writes O^T rows straight into ot_all (no PE cycles,
    no PSUM bank).
  - Output projection per i-tile (released as its last pair normalizes)
    accumulates over pairs from ot_all @ wo; y is DMA'd per half-row.
  - PE filler work (next pair's q/k projections + V tiles, output
    projections) is paced linearly across each pair's iterations so the
    exp-bound inner loop always has PE work to hide under.
PSUM budget (8 banks): fillers 2 + scores 2x2 + AV accum 2.
"""

import os
import sys
from collections import deque
from contextlib import ExitStack

import numpy as np

for _p in ("/opt/trn_rl_repo",):
    if os.path.isdir(_p) and _p not in sys.path:
        sys.path.insert(0, _p)

import ml_dtypes

import concourse.bacc as bacc
import concourse.mybir as mybir
import concourse.tile as tile
from concourse.bass_utils import run_bass_kernel_spmd
from concourse.masks import make_lower_triangular

AF = mybir.ActivationFunctionType
F32 = mybir.dt.float32
BF16 = mybir.dt.bfloat16
BF16NP = ml_dtypes.bfloat16

B, N, D, H, E = 4, 2048, 1024, 16, 64
N_CORES = 8
HPC = H // 2  # heads per core (tensor-parallel over 2 head groups)
CHW = 512     # i-chunk width (4 i-tiles)
VB = 65       # vna block per head: [V(64) | ones(1)]


def emit_attention(ctx, tc, y, xt, wq, wk, wv, wo, n, d, hpc, keep01):
    nc = tc.nc
    KT = d // 128        # contraction tiles for projections
    NT = n // 128        # j-tiles
    NCH = n // CHW       # i-chunks
    NP = hpc // 2        # head pairs
    DQ = hpc * 64        # per-core q/k/v width
    OC = min(512, d)     # out-projection column chunk
    NOC = d // OC
    TPC = CHW // 128     # i-tiles per chunk (2)
    DT = DQ // 128       # k-tiles for out projection
    VT = hpc * VB        # vna cols per j-tile (520)
    PW = TPC * VB        # po cols per head (130)

    # persistent SBUF tensors
    big = ctx.enter_context(tc.tile_pool(name="big", bufs=1))
    xt_sb = big.tile([128, KT * n], BF16, tag="xt", name="xt_sb")
    wq_sb = big.tile([128, KT * DQ], BF16, tag="wq", name="wq_sb")
    wk_sb = big.tile([128, KT * DQ], BF16, tag="wk", name="wk_sb")
    wv_sb = big.tile([128, KT * DQ], BF16, tag="wv", name="wv_sb")
    wo_sb = big.tile([128, DT * d], BF16, tag="wo", name="wo_sb")
    vna = big.tile([128, NT * VT], BF16, tag="vna", name="vna")
    ot_all = [big.tile([128, n], BF16, tag=f"ot{p_}", name=f"ot{p_}")
              for p_ in range(NP)]

    # input DMAs in consumption order, batched as one 3D-AP DMA per
    # (tensor, priority group) to minimize SP sequencer issue time:
    # xt last-512-chunk + pair-0 weight columns, then per remaining
    # 512-chunk: xt chunk + next pair's weight columns, then wo.
    ccl = n - 512
    xt_v = xt_sb.rearrange("p (k c) -> p k c", c=n)
    xd_v = xt.rearrange("(k p) c -> p k c", p=128)
    wviews = [
        (wq_sb.rearrange("p (k c) -> p k c", c=DQ),
         wq.rearrange("(k p) c -> p k c", p=128)),
        (wk_sb.rearrange("p (k c) -> p k c", c=DQ),
         wk.rearrange("(k p) c -> p k c", p=128)),
        (wv_sb.rearrange("p (k c) -> p k c", c=DQ),
         wv.rearrange("(k p) c -> p k c", p=128)),
    ]
    nc.sync.dma_start(wviews[0][0][:, :, 0:128], wviews[0][1][:, :, 0:128])
    nc.sync.dma_start(xt_v[:, 0:4, ccl:n], xd_v[:, 0:4, ccl:n])
    nc.sync.dma_start(wviews[1][0][:, :, 0:128], wviews[1][1][:, :, 0:128])
    nc.sync.dma_start(xt_v[:, 4:8, ccl:n], xd_v[:, 4:8, ccl:n])
    nc.sync.dma_start(wviews[2][0][:, :, 0:128], wviews[2][1][:, :, 0:128])
    for cd in range(ccl - 512, -512, -512):
        pp = (ccl - 512 - cd) // 512 + 1  # head pair whose weights go next
        nc.sync.dma_start(xt_v[:, :, cd : cd + 512], xd_v[:, :, cd : cd + 512])
        for wsv, wdv in wviews:
            nc.sync.dma_start(
                wsv[:, :, pp * 128 : (pp + 1) * 128],
                wdv[:, :, pp * 128 : (pp + 1) * 128],
            )
    nc.sync.dma_start(
        wo_sb.rearrange("p (t c) -> p t c", c=d),
        wo.rearrange("(t p) c -> p t c", p=128),
    )

    # ones columns of vna (col 64 of each per-head 65-block)
    nc.vector.memset(
        vna.rearrange("p (t h c) -> p t h c", h=hpc, c=VB)[:, :, :, 64:65], 1.0
    )

    # working pools
    qkvp = ctx.enter_context(tc.tile_pool(name="qkv", bufs=2))
    ptp = ctx.enter_context(tc.tile_pool(name="pt", bufs=4))
    onp = ctx.enter_context(tc.tile_pool(name="on", bufs=6))
    zp = ctx.enter_context(tc.tile_pool(name="z", bufs=4))
    ysp = ctx.enter_context(tc.tile_pool(name="yst", bufs=3))
    # PSUM budget (8 banks): fillers 2 + scores 2x2 + AV accum 2
    # (scores: one bank per head -- matmul output at a non-bank-aligned
    #  PSUM offset with tile_position (64, 0) faults on real hardware)
    psb = ctx.enter_context(tc.tile_pool(name="psb", bufs=2, space="PSUM"))
    pss = ctx.enter_context(tc.tile_pool(name="pss", bufs=2, space="PSUM"))
    pvp = ctx.enter_context(tc.tile_pool(name="pv", bufs=1, space="PSUM"))

    # PE p-state warmup: dummy matmuls on the const mask while the first
    # input DMAs stream in, so real work starts at full clock (the PE ramps
    # 0.65 -> 1.2 -> 2.4 GHz over ~3us of continuous busy)
    wps = psb.tile([128, 512], F32, tag="fill", name="warm")
    for i_ in range(20):
        nc.tensor.matmul(
            wps[:, 0:128], lhsT=keep01, rhs=keep01,
            start=(i_ == 0), stop=(i_ == 19),
        )

    def emit_v_chunk(t_, vp):
        """V natural layout for head pair vp, j-tile t_ (V cols of vna)."""
        ps = psb.tile([128, 128], F32, tag="fill", name="ps_v")
        for k_ in range(KT):
            nc.tensor.matmul(
                ps,
                lhsT=xt_sb[:, k_ * n + t_ * 128 : k_ * n + (t_ + 1) * 128],
                rhs=wv_sb[:, k_ * DQ + vp * 128 : k_ * DQ + (vp + 1) * 128],
                start=(k_ == 0),
                stop=(k_ == KT - 1),
            )
        nc.vector.tensor_copy(
            vna[:, t_ * VT + 2 * vp * VB : t_ * VT + (2 * vp + 2) * VB].rearrange(
                "p (h c) -> p h c", c=VB
            )[:, :, 0:64],
            ps.rearrange("p (h c) -> p h c", c=64),
        )

    def emit_qk_chunk(w_sb, p_, chn, dst):
        """One 512-wide n-chunk of the transposed Q or K projection."""
        ps = psb.tile([128, 512], F32, tag="fill", name="ps_qk")
        for k_ in range(KT):
            nc.tensor.matmul(
                ps,
                lhsT=w_sb[:, k_ * DQ + p_ * 128 : k_ * DQ + (p_ + 1) * 128],
                rhs=xt_sb[:, k_ * n + chn * 512 : k_ * n + chn * 512 + 512],
                start=(k_ == 0),
                stop=(k_ == KT - 1),
            )
        nc.vector.tensor_copy(dst[:, chn * 512 : (chn + 1) * 512], ps)

    ys_of = {}

    def emit_proj_half(it, hf):
        """Half of the output projection for one 128-row i-tile."""
        if hf == 0:
            ys_of[it] = ysp.tile([128, d], BF16, tag="y", name="ys")
        ys = ys_of[it]
        pf = psb.tile([128, OC], F32, tag="fill", name="pf")
        for p_ in range(NP):
            nc.tensor.matmul(
                pf,
                lhsT=ot_all[p_][:, it * 128 : (it + 1) * 128],
                rhs=wo_sb[:, p_ * d + hf * OC : p_ * d + hf * OC + OC],
                start=(p_ == 0),
                stop=(p_ == NP - 1),
            )
        nc.vector.tensor_copy(ys[:, hf * OC : (hf + 1) * OC], pf)
        nc.sync.dma_start(
            y[it * 128 : (it + 1) * 128, hf * OC : (hf + 1) * OC],
            ys[:, hf * OC : (hf + 1) * OC],
        )
        if hf == NOC - 1:
            del ys_of[it]

    filler = deque()

    def pump(k=1):
        for _ in range(min(k, len(filler))):
            filler.popleft()()

    # prelude: pair-0 q^T/k^T for the last 512-wide n-chunk, V j-tiles 15..14
    qt = qkvp.tile([128, n], BF16, tag="qt", name="qt0")
    kt = qkvp.tile([128, n], BF16, tag="kt", name="kt0")
    emit_qk_chunk(wq_sb, 0, n // 512 - 1, qt)
    emit_qk_chunk(wk_sb, 0, n // 512 - 1, kt)
    for t_ in range(NT - 1, NT - 1 - TPC, -1):
        emit_v_chunk(t_, 0)

    for p_ in range(NP):
        if p_ < NP - 1:
            # queue next pair's projections as PE filler work
            qt_n = qkvp.tile([128, n], BF16, tag="qt", name=f"qt{p_ + 1}")
            kt_n = qkvp.tile([128, n], BF16, tag="kt", name=f"kt{p_ + 1}")
            for chn in range(n // 512 - 1, -1, -1):
                filler.append(
                    lambda c=chn, t=qt_n, pp=p_ + 1: emit_qk_chunk(wq_sb, pp, c, t)
                )
                filler.append(
                    lambda c=chn, t=kt_n, pp=p_ + 1: emit_qk_chunk(wk_sb, pp, c, t)
                )
                for t_ in range(4 * chn + 3, 4 * chn - 1, -1):
                    filler.append(lambda tt=t_, pp=p_ + 1: emit_v_chunk(tt, pp))
            qk_next = (qt_n, kt_n)

        ps_of = {}
        SB = 512  # head-B bank offset inside the scores tile

        def emit_scores(cc, t_):
            o = 128 * t_ - CHW * cc
            w = min(CHW, o + 128)
            ps = pss.tile([128, 2 * SB], F32, tag="s", name="ps_s")
            nc.tensor.matmul(
                ps[:, :w],
                lhsT=qt[0:64, t_ * 128 : (t_ + 1) * 128],
                rhs=kt[0:64, cc * CHW : cc * CHW + w],
                start=True,
                stop=True,
            )
            nc.tensor.matmul(
                ps[:, SB : SB + w],
                lhsT=qt[64:128, t_ * 128 : (t_ + 1) * 128],
                rhs=kt[64:128, cc * CHW : cc * CHW + w],
                start=True,
                stop=True,
            )
            ps_of[(cc, t_)] = ps

        iters = [
            (cc, t_)
            for cc in range(NCH - 1, -1, -1)
            for t_ in range(NT - 1, TPC * cc - 1, -1)
        ]
        po_of = [None, None]

        def make_av(cc, t_, pab):
            o = 128 * t_ - CHW * cc
            w = min(CHW, o + 128)
            first = t_ == NT - 1

            def av():
                if first:
                    po_of[0] = pvp.tile([128, PW], F32, tag="poA", name="poA")
                    po_of[1] = pvp.tile([128, PW], F32, tag="poB", name="poB")
                # only the first matmul into each po bank uses start=True:
                # it marks the whole 2KB zero region pending, so the other
                # slices' first writes replace-by-pending; a start=True per
                # slice would re-mark earlier slices' bytes and drop them.
                for it in range(w // 128):
                    stop_it = t_ == TPC * cc + it
                    for h_ in (0, 1):
                        nc.tensor.matmul(
                            po_of[h_][:, it * VB : (it + 1) * VB],
                            lhsT=pab[:, h_ * CHW + it * 128
                                     : h_ * CHW + (it + 1) * 128],
                            rhs=vna[:, t_ * VT + (2 * p_ + h_) * VB
                                    : t_ * VT + (2 * p_ + h_ + 1) * VB],
                            start=first and it == 0,
                            stop=stop_it,
                            skip_group_check=True,
                        )

            return av

        def emit_norm_it(cc, it, poA, poB):
            """Normalize i-tile by Z, DMA-transpose o_nat into ot_all (O^T),
            and release its output projection once all pairs are done."""
            zz = zp.tile([128, 2, 1], F32, tag="zz", name="zz")
            nc.vector.reciprocal_approx_fast(
                zz[:, 0, :], poA[:, it * VB + 64 : (it + 1) * VB]
            )
            nc.vector.reciprocal_approx_fast(
                zz[:, 1, :], poB[:, it * VB + 64 : (it + 1) * VB]
            )
            on = onp.tile([128, 128], BF16, tag="on", name="on")
            nc.vector.tensor_scalar_mul(
                on[:, 0:64], poA[:, it * VB : it * VB + 64], zz[:, 0, :]
            )
            nc.vector.tensor_scalar_mul(
                on[:, 64:128], poB[:, it * VB : it * VB + 64], zz[:, 1, :]
            )
            nc.sync.dma_start_transpose(
                ot_all[p_][:, cc * CHW + it * 128 : cc * CHW + (it + 1) * 128],
                on,
            )
            if p_ == NP - 1:
                ita = TPC * cc + it
                for hf in range(NOC):
                    filler.append(lambda i=ita, h=hf: emit_proj_half(i, h))

        pending_av = None
        nf0 = len(filler) if p_ < NP - 1 else NOC * NT
        nit = len(iters)
        for i_, (cc, t_) in enumerate(iters):
            o = 128 * t_ - CHW * cc
            w = min(CHW, o + 128)
            last = t_ == TPC * cc
            idx = NT - 1 - t_
            if i_ == 0:
                emit_scores(cc, t_)
            ps = ps_of.pop((cc, t_))
            pab = ptp.tile([128, 2 * CHW], BF16, tag="pab", name="pab")
            nc.scalar.activation(
                pab.rearrange("p (a c) -> p a c", a=2)[:, :, 0:w],
                ps.rearrange("p (a c) -> p a c", a=2)[:, :, 0:w],
                AF.Exp,
            )
            if o < CHW:
                # diagonal j-tile: zero the strictly-masked (i > j) exp
                # entries with a 0/1 lower-triangular multiply on DVE
                # (cheaper than -1e9 tri-accumulate matmuls on PE)
                nc.vector.tensor_mul(
                    pab[:, o : o + 128], pab[:, o : o + 128], keep01
                )
                nc.vector.tensor_mul(
                    pab[:, CHW + o : CHW + o + 128],
                    pab[:, CHW + o : CHW + o + 128], keep01,
                )
            # keep ScalarE fed: next iteration's scores go ahead of AV/fillers
            if i_ + 1 < len(iters):
                emit_scores(*iters[i_ + 1])
            # PE work while ScalarE runs exp: jit prelude for pair 0,
            # queued fillers otherwise
            if p_ == 0 and cc > 0 and idx < TPC:
                emit_v_chunk(TPC * cc - 1 - idx, 0)
                if idx < 2:
                    # next 512-wide q/k chunk, finished before the last
                    # iter of this chunk emits the next chunk's scores
                    w_sb, dst = ((wq_sb, qt), (wk_sb, kt))[idx]
                    emit_qk_chunk(w_sb, 0, cc * CHW // 512 - 1, dst)
            elif filler and (
                p_ == 0
                or (i_ * nf0) // max(nit, 1) >= nf0 - len(filler)
            ):
                pump(1)
            # AV runs one slot behind so it never waits on this slot's exp
            if pending_av is not None:
                pending_av()
                itp = t_ + 1 - TPC * cc
                if 0 <= itp < TPC:
                    emit_norm_it(cc, itp, po_of[0], po_of[1])
            pending_av = make_av(cc, t_, pab)
            if not last:
                continue
            pending_av()
            pending_av = None
            emit_norm_it(cc, 0, po_of[0], po_of[1])

        if p_ < NP - 1:
            pump(len(filler))  # safety drain before the pair that needs them
            qt, kt = qk_next
    pump(len(filler))


def build_nc(n=N, d=D, hpc=HPC, num_devices=N_CORES, enable_asserts=False,
             reps=1):
    nc = bacc.Bacc(
        "TRN2",
        target_bir_lowering=False,
        debug=False,
        enable_asserts=enable_asserts,
        num_devices=num_devices,
    )
    dq = hpc * 64
    xt = nc.dram_tensor("xt", [d, n], BF16, kind="ExternalInput").ap()
    wq = nc.dram_tensor("wq", [d, dq], BF16, kind="ExternalInput").ap()
    wk = nc.dram_tensor("wk", [d, dq], BF16, kind="ExternalInput").ap()
    wv = nc.dram_tensor("wv", [d, dq], BF16, kind="ExternalInput").ap()
    wo = nc.dram_tensor("wo", [dq, d], BF16, kind="ExternalInput").ap()
    y = nc.dram_tensor("y", [n, d], BF16, kind="ExternalOutput").ap()
    with tile.TileContext(nc) as tc:
        with ExitStack() as cctx:
            cpool = cctx.enter_context(tc.tile_pool(name="consts", bufs=1))
            # keep-mask for the diagonal block: 1 where i <= j, else 0
            keep01 = cpool.tile([128, 128], BF16, tag="keep", name="keep01")
            make_lower_triangular(nc, keep01, val=1.0, diag=True)
            for _rep in range(reps):
                with ExitStack() as ctx:
                    emit_attention(ctx, tc, y, xt, wq, wk, wv, wo, n, d, hpc,
                                   keep01)
    nc.compile()
    return nc


def make_in_maps(x, W_qkv, W_o):
    scale = np.float32(1.0 / np.sqrt(E))
    dq = HPC * 64
    in_maps = []
    for c in range(N_CORES):
        b, g = divmod(c, 2)
        in_maps.append(
            {
                "xt": np.ascontiguousarray(x[b].T).astype(BF16NP),
                "wq": (W_qkv[:, g * dq : (g + 1) * dq] * scale).astype(BF16NP),
                "wk": np.ascontiguousarray(
                    W_qkv[:, D + g * dq : D + (g + 1) * dq]
                ).astype(BF16NP),
                "wv": np.ascontiguousarray(
                    W_qkv[:, 2 * D + g * dq : 2 * D + (g + 1) * dq]
                ).astype(BF16NP),
                "wo": np.ascontiguousarray(W_o[g * dq : (g + 1) * dq, :]).astype(
                    BF16NP
                ),
            }
        )
    return in_maps


_NC_CACHE = {}


def kernel(x, W_qkv, W_o):
    x = np.asarray(x, dtype=np.float32)
    W_qkv = np.asarray(W_qkv, dtype=np.float32)
    W_o = np.asarray(W_o, dtype=np.float32)
    if "nc" not in _NC_CACHE:
        _NC_CACHE["nc"] = build_nc()
    in_maps = make_in_maps(x, W_qkv, W_o)
    res = run_bass_kernel_spmd(_NC_CACHE["nc"], in_maps, list(range(N_CORES)))
    ys = [np.asarray(res.results[i]["y"], dtype=np.float32) for i in range(N_CORES)]
    return np.stack([ys[2 * b] + ys[2 * b + 1] for b in range(B)])


# revision 63
# speedup vs baseline: 1.0573x; 1.0573x over previous
"""Trainium2 Bass kernel: multi-head attention (transposed-causal softmax).

Reference math (B=4, N=2048, D=1024, H=16, E=64):
    qkv = x @ W_qkv -> split (3, H, E)
    scores[i, j] = k_i . q_j / sqrt(E)          (i = key pos, j = query pos)
    mask: keep i <= j; softmax over j; out[i] = sum_j attn[i, j] v_j
    y = concat_heads(out) @ W_o
Sharding (8 cores): data-parallel over batch (4) x tensor-parallel over
head-groups (2 groups of 8 heads); the host sums the two partial
projections per batch.

Per-core dataflow (v9, CoreSim ~208us vs ~279us for the v2 baseline):
  - xt [D, N] supplied transposed so projections contract D on partitions.
  - Inputs stream in consumption order as ONE batched 3D-AP DMA per
    (tensor, priority group) -- the SP sequencer's ~565ns per-DMA issue
    cost made many small DMAs the startup bottleneck.
  - V is projected per head pair into vna [128, NT*520]: per j-tile,
    8 x [64 V cols | 1 ones col]; the ones column makes the AV matmul
    emit the softmax denominator Z alongside O.  Pair 0's V tiles are
    computed just-in-time; pairs 1-3's V are deferred PE filler.
  - i-chunks are 512 wide.  Scores land in [128, 2x512] PSUM (one BANK
    per head -- a matmul writing a non-bank-aligned PSUM offset with
    tile_position (64, 0) faults on real hardware), double-buffered;
    one exp per (chunk, j-tile) covers both heads.  The causal mask on
    the diagonal block is a 0/1 lower-triangular multiply on DVE after
    exp (cheaper than -1e9 tri-accumulate matmuls on PE).
  - AV runs in NATURAL layout: per 128-row i-tile, lhsT = exp-scores
    slice [j, i-tile], rhs = vna block [j, 65] -> po[i, 65] accumulated
    over j-tiles (65-col streams instead of w<=512 per head, ~2x less PE
    than the O^T form).  PSUM zero-region discipline: only the first
    matmul into a po bank uses start=True; a start=True per slice would
    re-mark earlier slices' bytes pending-zero and drop them.
  - Normalization per i-tile as soon as its diagonal j-tile stops:
    reciprocal_approx_fast on the Z column, per-partition
    tensor_scalar_mul -> o_nat [128, 128] bf16 (2 heads), then a DMA
    XBAR transpose writes O^T rows straight into ot_all (no PE cycles,
    no PSUM bank).
  - Output projection per i-tile (released as its last pair normalizes)
    accumulates over pairs from ot_all @ wo; y is DMA'd per half-row.
  - PE filler work (next pair's q/k projections + V tiles, output
    projections) is paced linearly across each pair's iterations so the
    exp-bound inner loop always has PE work to hide under.  Dummy
    matmuls at t=0 ramp the PE p-state while the first DMAs stream.
PSUM budget (8 banks): fillers 2 + scores 2x2 + AV accum 2.
"""

import os
import sys
from collections import deque
from contextlib import ExitStack

import numpy as np

for _p in ("/opt/trn_rl_repo",):
    if os.path.isdir(_p) and _p not in sys.path:
        sys.path.insert(0, _p)

import ml_dtypes

import concourse.bacc as bacc
import concourse.mybir as mybir
import concourse.tile as tile
from concourse.bass_utils import run_bass_kernel_spmd
from concourse.masks import make_lower_triangular

AF = mybir.ActivationFunctionType
F32 = mybir.dt.float32
BF16 = mybir.dt.bfloat16
BF16NP = ml_dtypes.bfloat16

B, N, D, H, E = 4, 2048, 1024, 16, 64
N_CORES = 8
HPC = H // 2  # heads per core (tensor-parallel over 2 head groups)
CHW = 512     # i-chunk width (4 i-tiles)
VB = 65       # vna block per head: [V(64) | ones(1)]


def emit_attention(ctx, tc, y, xt, wq, wk, wv, wo, n, d, hpc, keep01):
    nc = tc.nc
    KT = d // 128        # contraction tiles for projections
    NT = n // 128        # j-tiles
    NCH = n // CHW       # i-chunks
    NP = hpc // 2        # head pairs
    DQ = hpc * 64        # per-core q/k/v width
    OC = min(512, d)     # out-projection column chunk
    NOC = d // OC
    TPC = CHW // 128     # i-tiles per chunk (2)
    DT = DQ // 128       # k-tiles for out projection
    VT = hpc * VB        # vna cols per j-tile (520)
    PW = TPC * VB        # po cols per head (130)

    # persistent SBUF tensors
    big = ctx.enter_context(tc.tile_pool(name="big", bufs=1))
    xt_sb = big.tile([128, KT * n], BF16, tag="xt", name="xt_sb")
    wq_sb = big.tile([128, KT * DQ], BF16, tag="wq", name="wq_sb")
    wk_sb = big.tile([128, KT * DQ], BF16, tag="wk", name="wk_sb")
    wv_sb = big.tile([128, KT * DQ], BF16, tag="wv", name="wv_sb")
    wo_sb = big.tile([128, DT * d], BF16, tag="wo", name="wo_sb")
    vna = big.tile([128, NT * VT], BF16, tag="vna", name="vna")
    ot_all = [big.tile([128, n], BF16, tag=f"ot{p_}", name=f"ot{p_}")
              for p_ in range(NP)]

    # input DMAs in consumption order, batched as one 3D-AP DMA per
    # (tensor, priority group) to minimize SP sequencer issue time:
    # xt last-512-chunk + pair-0 weight columns, then per remaining
    # 512-chunk: xt chunk + next pair's weight columns, then wo.
    ccl = n - 512
    xt_v = xt_sb.rearrange("p (k c) -> p k c", c=n)
    xd_v = xt.rearrange("(k p) c -> p k c", p=128)
    wviews = [
        (wq_sb.rearrange("p (k c) -> p k c", c=DQ),
         wq.rearrange("(k p) c -> p k c", p=128)),
        (wk_sb.rearrange("p (k c) -> p k c", c=DQ),
         wk.rearrange("(k p) c -> p k c", p=128)),
        (wv_sb.rearrange("p (k c) -> p k c", c=DQ),
         wv.rearrange("(k p) c -> p k c", p=128)),
    ]
    nc.sync.dma_start(wviews[0][0][:, :, 0:128], wviews[0][1][:, :, 0:128])
    nc.sync.dma_start(xt_v[:, 0:4, ccl:n], xd_v[:, 0:4, ccl:n])
    nc.sync.dma_start(wviews[1][0][:, :, 0:128], wviews[1][1][:, :, 0:128])
    nc.sync.dma_start(xt_v[:, 4:8, ccl:n], xd_v[:, 4:8, ccl:n])
    nc.sync.dma_start(wviews[2][0][:, :, 0:128], wviews[2][1][:, :, 0:128])
    for cd in range(ccl - 512, -512, -512):
        pp = (ccl - 512 - cd) // 512 + 1  # head pair whose weights go next
        nc.sync.dma_start(xt_v[:, :, cd : cd + 512], xd_v[:, :, cd : cd + 512])
        for wsv, wdv in wviews:
            nc.sync.dma_start(
                wsv[:, :, pp * 128 : (pp + 1) * 128],
                wdv[:, :, pp * 128 : (pp + 1) * 128],
            )
    nc.sync.dma_start(
        wo_sb.rearrange("p (t c) -> p t c", c=d),
        wo.rearrange("(t p) c -> p t c", p=128),
    )

    # ones columns of vna (col 64 of each per-head 65-block)
    nc.vector.memset(
        vna.rearrange("p (t h c) -> p t h c", h=hpc, c=VB)[:, :, :, 64:65], 1.0
    )

    # working pools
    qkvp = ctx.enter_context(tc.tile_pool(name="qkv", bufs=2))
    ptp = ctx.enter_context(tc.tile_pool(name="pt", bufs=4))
    onp = ctx.enter_context(tc.tile_pool(name="on", bufs=6))
    zp = ctx.enter_context(tc.tile_pool(name="z", bufs=4))
    ysp = ctx.enter_context(tc.tile_pool(name="yst", bufs=3))
    # PSUM budget (8 banks): fillers 2 + scores 2x2 + AV accum 2
    # (scores: one bank per head -- matmul output at a non-bank-aligned
    #  PSUM offset with tile_position (64, 0) faults on real hardware)
    psb = ctx.enter_context(tc.tile_pool(name="psb", bufs=2, space="PSUM"))
    pss = ctx.enter_context(tc.tile_pool(name="pss", bufs=2, space="PSUM"))
    pvp = ctx.enter_context(tc.tile_pool(name="pv", bufs=1, space="PSUM"))

    # PE p-state warmup: dummy matmuls on the const mask while the first
    # input DMAs stream in, so real work starts at full clock (the PE ramps
    # 0.65 -> 1.2 -> 2.4 GHz over ~3us of continuous busy)
    wps = psb.tile([128, 512], F32, tag="fill", name="warm")
    for i_ in range(20):
        nc.tensor.matmul(
            wps[:, 0:128], lhsT=keep01, rhs=keep01,
            start=(i_ == 0), stop=(i_ == 19),
        )

    def emit_v_chunk(t_, vp):
        """V natural layout for head pair vp, j-tile t_ (V cols of vna)."""
        ps = psb.tile([128, 128], F32, tag="fill", name="ps_v")
        for k_ in range(KT):
            nc.tensor.matmul(
                ps,
                lhsT=xt_sb[:, k_ * n + t_ * 128 : k_ * n + (t_ + 1) * 128],
                rhs=wv_sb[:, k_ * DQ + vp * 128 : k_ * DQ + (vp + 1) * 128],
                start=(k_ == 0),
                stop=(k_ == KT - 1),
            )
        nc.vector.tensor_copy(
            vna[:, t_ * VT + 2 * vp * VB : t_ * VT + (2 * vp + 2) * VB].rearrange(
                "p (h c) -> p h c", c=VB
            )[:, :, 0:64],
            ps.rearrange("p (h c) -> p h c", c=64),
        )

    def emit_qk_chunk(w_sb, p_, chn, dst):
        """One 512-wide n-chunk of the transposed Q or K projection."""
        ps = psb.tile([128, 512], F32, tag="fill", name="ps_qk")
        for k_ in range(KT):
            nc.tensor.matmul(
                ps,
                lhsT=w_sb[:, k_ * DQ + p_ * 128 : k_ * DQ + (p_ + 1) * 128],
                rhs=xt_sb[:, k_ * n + chn * 512 : k_ * n + chn * 512 + 512],
                start=(k_ == 0),
                stop=(k_ == KT - 1),
            )
        nc.vector.tensor_copy(dst[:, chn * 512 : (chn + 1) * 512], ps)

    ys_of = {}

    def emit_proj_half(it, hf):
        """Half of the output projection for one 128-row i-tile."""
        if hf == 0:
            ys_of[it] = ysp.tile([128, d], BF16, tag="y", name="ys")
        ys = ys_of[it]
        pf = psb.tile([128, OC], F32, tag="fill", name="pf")
        for p_ in range(NP):
            nc.tensor.matmul(
                pf,
                lhsT=ot_all[p_][:, it * 128 : (it + 1) * 128],
                rhs=wo_sb[:, p_ * d + hf * OC : p_ * d + hf * OC + OC],
                start=(p_ == 0),
                stop=(p_ == NP - 1),
            )
        nc.vector.tensor_copy(ys[:, hf * OC : (hf + 1) * OC], pf)
        nc.sync.dma_start(
            y[it * 128 : (it + 1) * 128, hf * OC : (hf + 1) * OC],
            ys[:, hf * OC : (hf + 1) * OC],
        )
        if hf == NOC - 1:
            del ys_of[it]

    filler = deque()

    def pump(k=1):
        for _ in range(min(k, len(filler))):
            filler.popleft()()

    # prelude: pair-0 q^T/k^T for the last 512-wide n-chunk, V j-tiles 15..14
    qt = qkvp.tile([128, n], BF16, tag="qt", name="qt0")
    kt = qkvp.tile([128, n], BF16, tag="kt", name="kt0")
    emit_qk_chunk(wq_sb, 0, n // 512 - 1, qt)
    emit_qk_chunk(wk_sb, 0, n // 512 - 1, kt)
    for t_ in range(NT - 1, NT - 1 - TPC, -1):
        emit_v_chunk(t_, 0)

    for p_ in range(NP):
        if p_ < NP - 1:
            # queue next pair's projections as PE filler work
            qt_n = qkvp.tile([128, n], BF16, tag="qt", name=f"qt{p_ + 1}")
            kt_n = qkvp.tile([128, n], BF16, tag="kt", name=f"kt{p_ + 1}")
            for chn in range(n // 512 - 1, -1, -1):
                filler.append(
                    lambda c=chn, t=qt_n, pp=p_ + 1: emit_qk_chunk(wq_sb, pp, c, t)
                )
                filler.append(
                    lambda c=chn, t=kt_n, pp=p_ + 1: emit_qk_chunk(wk_sb, pp, c, t)
                )
                for t_ in range(4 * chn + 3, 4 * chn - 1, -1):
                    filler.append(lambda tt=t_, pp=p_ + 1: emit_v_chunk(tt, pp))
            qk_next = (qt_n, kt_n)

        ps_of = {}
        SB = 512  # head-B bank offset inside the scores tile

        def emit_scores(cc, t_):
            o = 128 * t_ - CHW * cc
            w = min(CHW, o + 128)
            ps = pss.tile([128, 2 * SB], F32, tag="s", name="ps_s")
            nc.tensor.matmul(
                ps[:, :w],
                lhsT=qt[0:64, t_ * 128 : (t_ + 1) * 128],
                rhs=kt[0:64, cc * CHW : cc * CHW + w],
                start=True,
                stop=True,
            )
            nc.tensor.matmul(
                ps[:, SB : SB + w],
                lhsT=qt[64:128, t_ * 128 : (t_ + 1) * 128],
                rhs=kt[64:128, cc * CHW : cc * CHW + w],
                start=True,
                stop=True,
            )
            ps_of[(cc, t_)] = ps

        iters = [
            (cc, t_)
            for cc in range(NCH - 1, -1, -1)
            for t_ in range(NT - 1, TPC * cc - 1, -1)
        ]
        po_of = [None, None]

        def make_av(cc, t_, pab):
            o = 128 * t_ - CHW * cc
            w = min(CHW, o + 128)
            first = t_ == NT - 1

            def av():
                if first:
                    po_of[0] = pvp.tile([128, PW], F32, tag="poA", name="poA")
                    po_of[1] = pvp.tile([128, PW], F32, tag="poB", name="poB")
                # only the first matmul into each po bank uses start=True:
                # it marks the whole 2KB zero region pending, so the other
                # slices' first writes replace-by-pending; a start=True per
                # slice would re-mark earlier slices' bytes and drop them.
                for it in range(w // 128):
                    stop_it = t_ == TPC * cc + it
                    for h_ in (0, 1):
                        nc.tensor.matmul(
                            po_of[h_][:, it * VB : (it + 1) * VB],
                            lhsT=pab[:, h_ * CHW + it * 128
                                     : h_ * CHW + (it + 1) * 128],
                            rhs=vna[:, t_ * VT + (2 * p_ + h_) * VB
                                    : t_ * VT + (2 * p_ + h_ + 1) * VB],
                            start=first and it == 0,
                            stop=stop_it,
                            skip_group_check=True,
                        )

            return av

        def emit_norm_it(cc, it, poA, poB):
            """Normalize i-tile by Z, DMA-transpose o_nat into ot_all (O^T),
            and release its output projection once all pairs are done."""
            zz = zp.tile([128, 2, 1], F32, tag="zz", name="zz")
            nc.vector.reciprocal_approx_fast(
                zz[:, 0, :], poA[:, it * VB + 64 : (it + 1) * VB]
            )
            nc.vector.reciprocal_approx_fast(
                zz[:, 1, :], poB[:, it * VB + 64 : (it + 1) * VB]
            )
            on = onp.tile([128, 128], BF16, tag="on", name="on")
            nc.vector.tensor_scalar_mul(
                on[:, 0:64], poA[:, it * VB : it * VB + 64], zz[:, 0, :]
            )
            nc.vector.tensor_scalar_mul(
                on[:, 64:128], poB[:, it * VB : it * VB + 64], zz[:, 1, :]
            )
            nc.sync.dma_start_transpose(
                ot_all[p_][:, cc * CHW + it * 128 : cc * CHW + (it + 1) * 128],
                on,
            )
            if p_ == NP - 1:
                ita = TPC * cc + it
                for hf in range(NOC):
                    filler.append(lambda i=ita, h=hf: emit_proj_half(i, h))

        pending_av = None
        nf0 = len(filler) if p_ < NP - 1 else NOC * NT
        nit = len(iters)
        for i_, (cc, t_) in enumerate(iters):
            o = 128 * t_ - CHW * cc
            w = min(CHW, o + 128)
            last = t_ == TPC * cc
            idx = NT - 1 - t_
            if i_ == 0:
                emit_scores(cc, t_)
            ps = ps_of.pop((cc, t_))
            pab = ptp.tile([128, 2 * CHW], BF16, tag="pab", name="pab")
            nc.scalar.activation(
                pab.rearrange("p (a c) -> p a c", a=2)[:, :, 0:w],
                ps.rearrange("p (a c) -> p a c", a=2)[:, :, 0:w],
                AF.Exp,
            )
            if o < CHW:
                # diagonal j-tile: zero the strictly-masked (i > j) exp
                # entries with a 0/1 lower-triangular multiply on DVE
                # (cheaper than -1e9 tri-accumulate matmuls on PE)
                nc.vector.tensor_mul(
                    pab[:, o : o + 128], pab[:, o : o + 128], keep01
                )
                nc.vector.tensor_mul(
                    pab[:, CHW + o : CHW + o + 128],
                    pab[:, CHW + o : CHW + o + 128], keep01,
                )
            # keep ScalarE fed: next iteration's scores go ahead of AV/fillers
            if i_ + 1 < len(iters):
                emit_scores(*iters[i_ + 1])
            # PE work while ScalarE runs exp: jit prelude for pair 0,
            # queued fillers otherwise
            if p_ == 0 and cc > 0 and idx < TPC:
                emit_v_chunk(TPC * cc - 1 - idx, 0)
                if idx < 2:
                    # next 512-wide q/k chunk, finished before the last
                    # iter of this chunk emits the next chunk's scores
                    w_sb, dst = ((wq_sb, qt), (wk_sb, kt))[idx]
                    emit_qk_chunk(w_sb, 0, cc * CHW // 512 - 1, dst)
            elif filler and (
                p_ == 0
                or (i_ * nf0) // max(nit, 1) >= nf0 - len(filler)
            ):
                pump(1)
            # AV runs one slot behind so it never waits on this slot's exp
            if pending_av is not None:
                pending_av()
                itp = t_ + 1 - TPC * cc
                if 0 <= itp < TPC:
                    emit_norm_it(cc, itp, po_of[0], po_of[1])
            pending_av = make_av(cc, t_, pab)
            if not last:
                continue
            pending_av()
            pending_av = None
            emit_norm_it(cc, 0, po_of[0], po_of[1])

        if p_ < NP - 1:
            pump(len(filler))  # safety drain before the pair that needs them
            qt, kt = qk_next
    pump(len(filler))


def build_nc(n=N, d=D, hpc=HPC, num_devices=N_CORES, enable_asserts=False,
             reps=1):
    nc = bacc.Bacc(
        "TRN2",
        target_bir_lowering=False,
        debug=False,
        enable_asserts=enable_asserts,
        num_devices=num_devices,
    )
    dq = hpc * 64
    xt = nc.dram_tensor("xt", [d, n], BF16, kind="ExternalInput").ap()
    wq = nc.dram_tensor("wq", [d, dq], BF16, kind="ExternalInput").ap()
    wk = nc.dram_tensor("wk", [d, dq], BF16, kind="ExternalInput").ap()
    wv = nc.dram_tensor("wv", [d, dq], BF16, kind="ExternalInput").ap()
    wo = nc.dram_tensor("wo", [dq, d], BF16, kind="ExternalInput").ap()
    y = nc.dram_tensor("y", [n, d], BF16, kind="ExternalOutput").ap()
    with tile.TileContext(nc) as tc:
        with ExitStack() as cctx:
            cpool = cctx.enter_context(tc.tile_pool(name="consts", bufs=1))
            # keep-mask for the diagonal block: 1 where i <= j, else 0
            keep01 = cpool.tile([128, 128], BF16, tag="keep", name="keep01")
            make_lower_triangular(nc, keep01, val=1.0, diag=True)
            for _rep in range(reps):
                with ExitStack() as ctx:
                    emit_attention(ctx, tc, y, xt, wq, wk, wv, wo, n, d, hpc,
                                   keep01)
    nc.compile()
    return nc


def make_in_maps(x, W_qkv, W_o):
    scale = np.float32(1.0 / np.sqrt(E))
    dq = HPC * 64
    in_maps = []
    for c in range(N_CORES):
        b, g = divmod(c, 2)
        in_maps.append(
            {
                "xt": np.ascontiguousarray(x[b].T).astype(BF16NP),
                "wq": (W_qkv[:, g * dq : (g + 1) * dq] * scale).astype(BF16NP),
                "wk": np.ascontiguousarray(
                    W_qkv[:, D + g * dq : D + (g + 1) * dq]
                ).astype(BF16NP),
                "wv": np.ascontiguousarray(
                    W_qkv[:, 2 * D + g * dq : 2 * D + (g + 1) * dq]
                ).astype(BF16NP),
                "wo": np.ascontiguousarray(W_o[g * dq : (g + 1) * dq, :]).astype(
                    BF16NP
                ),
            }
        )
    return in_maps


_NC_CACHE = {}


def kernel(x, W_qkv, W_o):
    x = np.asarray(x, dtype=np.float32)
    W_qkv = np.asarray(W_qkv, dtype=np.float32)
    W_o = np.asarray(W_o, dtype=np.float32)
    if "nc" not in _NC_CACHE:
        _NC_CACHE["nc"] = build_nc()
    in_maps = make_in_maps(x, W_qkv, W_o)
    res = run_bass_kernel_spmd(_NC_CACHE["nc"], in_maps, list(range(N_CORES)))
    ys = [np.asarray(res.results[i]["y"], dtype=np.float32) for i in range(N_CORES)]
    return np.stack([ys[2 * b] + ys[2 * b + 1] for b in range(B)])
